# revision 26
# baseline (speedup 1.0000x reference)
"""Multi-hot embedding bag kernel for Trainium2 (8 NeuronCores, batch-sharded).

Computes, for 5 feature groups g with multi-hot int32 matrices A_g [B, V_g]
and weights W_g [V_g, 64]:
    out = concat_g(norm_g(A_g @ W_g))  with the original module's quirks:
    - "decades" is normalized by its own row-sum AND by the movie row-sum
    - "movies" is never normalized
    - remaining groups are normalized by their own row-sum (rows with sum 0
      are left unnormalized)

Strategy per core (256 batch rows):
  - A is transposed on the HOST (int32 preserved) into [128, C, 256]: vocab
    on partitions, one 128-vocab chunk per 256-batch column block, all 5
    groups concatenated chunk-wise with zero padding. The kernel streams
    uniform 32-chunk slabs HBM->SBUF via gpsimd (SWDGE) DMA with an
    int32->fp16 cast ({0,1} exact; this stream is the roofline).
  - W is packed host-side per chunk as [W | 1] (the ones column accumulates
    the multi-hot row-sums): fp16 for movies (unnormalized output, sets the
    global scale), fp8e4m3 for the 4 normalized groups (quantization error
    is divided by ~row-sum, negligible; the PE takes mixed fp8 stationary x
    fp16 moving operands). W streams in group-aligned slabs of up to 128
    chunks, independent of the A slab grid.
  - per chunk, ONE matmul accumulates accT [65, 256] in PSUM (lhsT =
    W-chunk stationary, rhs = A^T-chunk moving); a group's chunks form one
    PSUM accumulation group.
  - at group end accT is copied to SBUF, transposed back on the PE (fp32
    identity) to [256, 65], then normalized with per-row reciprocals.
"""

import math

import numpy as np

import concourse.bass as bass
import concourse.tile as tile
from concourse import bacc, mybir
from concourse.bass_utils import run_bass_kernel_spmd
from concourse.masks import make_identity

B = 2048
LF = 64
FE = LF + 1  # weights + ones column
N_CORES = 8
BPC = B // N_CORES  # 256 batch rows per core
P = 128
SLAB = 32  # vocab chunks per A-slab DMA (32 -> 4 MiB int32 reads)
W_SLAB = 128  # vocab chunks per W-slab DMA (within a group)

_FP16 = mybir.dt.float16
_FP32 = mybir.dt.float32
_FP8 = mybir.dt.float8e4

# (key, idx input name, weight input name, vocab size, output column offset,
#  W dtype). Movies first so its row-sum reciprocal exists when decades is
# normalized.
GROUPS = [
    ("mov", "movie_idxs", "W_mov", 60000, 64, _FP16),
    ("dec", "decade_idxs", "W_dec", 12, 0, _FP8),
    ("cat", "category_idxs", "W_cat", 32, 128, _FP8),
    ("per", "person_idxs", "W_per", 100000, 192, _FP8),
    ("com", "company_idxs", "W_com", 20000, 256, _FP8),
]
N_CH = [math.ceil(v / P) for _, _, _, v, _, _ in GROUPS]
C_TOT = sum(N_CH)  # 1410 chunks of 128 vocab rows
C_16 = N_CH[0]  # fp16 (movie) chunks
C_8 = C_TOT - C_16  # fp8 chunks
OUT_COLS = 5 * LF

# static chunk -> group table and W slab list
_CHUNK_GROUP = []  # (group idx, group first chunk)
for _gi, _n in enumerate(N_CH):
    _CHUNK_GROUP += [(_gi, len(_CHUNK_GROUP))] * _n

# A slabs: uniform SLAB-chunk slices with a short taper at the stream end
# so the PE drains only a few chunks after the last DMA. Note: start/taper/
# tail DMA-idle totals ~17us across every geometry tried (uniform, 5-step
# taper, 3-step taper) — the PE drains at half speed (~213ns/matmul) after
# going idle, eating whatever the taper saves. Kept because it measured
# equal-best and bounds the drain length.
_TAIL = [24, 8, 2]
_A_SLABS = {}  # start chunk -> n chunks
_c = 0
while C_TOT - _c > sum(_TAIL):
    _A_SLABS[_c] = SLAB
    _c += SLAB
for _t in _TAIL:
    _A_SLABS[_c] = _t
    _c += _t
assert _c == C_TOT

# W slabs: (global start chunk -> n chunks, w-tensor chunk base, dtype).
# The very first W slab is small so A descriptors reach the DMA queues
# without a large W transfer queued ahead of them.
_W_SLABS = {}
_g0 = 0
_w8_base = 0
for _gi, _n in enumerate(N_CH):
    _dt = GROUPS[_gi][5]
    _starts = [0]
    if _gi == 0:
        _starts.append(min(SLAB, _n))
    while _starts[-1] < _n:
        _nxt = _starts[-1] + W_SLAB
        if _nxt >= _n:
            break
        _starts.append(_nxt)
    for _i, _s in enumerate(_starts):
        _end = _starts[_i + 1] if _i + 1 < len(_starts) else _n
        _ch = _end - _s
        _base = _s if _dt == _FP16 else _w8_base + _s
        _W_SLABS[_g0 + _s] = (_ch, _base, _dt)
    _g0 += _n
    if _dt != _FP16:
        _w8_base += _n


def _build() -> bass.Bass:
    nc = bacc.Bacc(None, target_bir_lowering=False)

    a_dram = nc.dram_tensor("a_all", [P, C_TOT * 2 * P], mybir.dt.int32,
                            kind="ExternalInput")
    w16_dram = nc.dram_tensor("w16", [P, C_16 * FE], _FP16,
                              kind="ExternalInput")
    w8_dram = nc.dram_tensor("w8", [P, C_8 * FE], _FP8, kind="ExternalInput")
    out = nc.dram_tensor("out", [BPC, OUT_COLS], _FP32, kind="ExternalOutput")

    with tile.TileContext(nc) as tc:
        with (
            tc.tile_pool(name="singles", bufs=1) as singles,
            tc.tile_pool(name="apool", bufs=6) as apool,
            tc.tile_pool(name="wpool", bufs=2) as wpool,
            tc.tile_pool(name="npool", bufs=4) as npool,
            tc.tile_pool(name="accp", bufs=2, space="PSUM") as accp,
            tc.tile_pool(name="backp", bufs=2, space="PSUM") as backp,
        ):
            ident32 = singles.tile([P, P], _FP32)

            out_sb = [singles.tile([P, OUT_COLS], _FP32, name=f"out_sb{i}")
                      for i in range(2)]
            rmov = [singles.tile([P, 1], _FP32, name=f"rmov{i}")
                    for i in range(2)]

            cur_acc = {}  # group key -> live PSUM accumulator tile
            a_sb = w_sb = None
            a_base = w_base = 0
            for cidx in range(C_TOT):
                if cidx in _A_SLABS:
                    ch = _A_SLABS[cidx]
                    a_sb = apool.tile([P, SLAB, 2 * P], _FP16, tag="a")
                    nc.gpsimd.dma_start(
                        a_sb[:, :ch, :],
                        a_dram[:, cidx * 2 * P:(cidx + ch) * 2 * P]
                        .rearrange("p (c b) -> p c b", b=2 * P),
                    )
                    a_base = cidx
                if cidx in _W_SLABS:
                    wch, wb, wdt = _W_SLABS[cidx]
                    wdram = w16_dram if wdt == _FP16 else w8_dram
                    w_sb = wpool.tile([P, W_SLAB, FE], wdt,
                                      tag=f"w{wdt.name}")
                    nc.sync.dma_start(
                        w_sb[:, :wch, :],
                        wdram[:, wb * FE:(wb + wch) * FE].rearrange(
                            "p (c f) -> p c f", f=FE),
                    )
                    w_base = cidx
                if cidx == 0:
                    # after the first DMA issuances so identity construction
                    # (gpsimd) doesn't delay the first SWDGE descriptors
                    make_identity(nc, ident32)

                gi, g0 = _CHUNK_GROUP[cidx]
                key, _, _, v, col, _ = GROUPS[gi]
                if cidx == g0:
                    cur_acc[key] = accp.tile([FE, 2 * P], _FP32, tag="acc",
                                             name=f"accT_{key}")
                nc.tensor.matmul(
                    cur_acc[key],
                    lhsT=w_sb[:, cidx - w_base, :],
                    rhs=a_sb[:, cidx - a_base, :],
                    start=(cidx == g0),
                    stop=(cidx == g0 + N_CH[gi] - 1),
                )
                if cidx != g0 + N_CH[gi] - 1:
                    continue

                # group epilogue: back-transpose, normalize, stage output
                accT_sb = npool.tile([FE, 2 * P], _FP32, tag="accsb")
                nc.vector.tensor_copy(accT_sb, cur_acc[key])
                for bt in range(2):
                    out2 = backp.tile([P, FE], _FP32, tag="out2")
                    nc.tensor.matmul(
                        out2,
                        lhsT=accT_sb[:, bass.ts(bt, P)],
                        rhs=ident32[:FE, :FE],
                        start=True, stop=True,
                    )
                    s = npool.tile([P, 1], _FP32, tag="s")
                    nc.vector.tensor_scalar_max(s, out2[:, LF:FE], 1.0)
                    nc.vector.reciprocal(s, s)
                    if key == "mov":
                        # movies are left unnormalized; stash 1/max(sum,1)
                        # for the decades double-normalization
                        nc.vector.tensor_copy(rmov[bt], s)
                        nc.scalar.copy(out_sb[bt][:, col:col + LF],
                                       out2[:, :LF])
                    else:
                        if key == "dec":
                            nc.vector.tensor_mul(s, s, rmov[bt])
                        nc.vector.tensor_scalar_mul(
                            out_sb[bt][:, col:col + LF], out2[:, :LF], s)

            for bt in range(2):
                nc.sync.dma_start(out[bt * P:(bt + 1) * P, :], out_sb[bt])

    nc.finalize()
    return nc


_NC_CACHE: bass.Bass | None = None


def _get_nc() -> bass.Bass:
    global _NC_CACHE
    if _NC_CACHE is None:
        _NC_CACHE = _build()
    return _NC_CACHE


def _pack_weights(inputs: dict) -> dict[str, np.ndarray]:
    """Chunk-major [W_g | 1] packs: fp16 for movies, fp8e4m3 for the rest."""
    w16 = np.zeros((P, C_16, FE), np.float16)
    w8 = np.zeros((P, C_8, FE), mybir.dt.np(_FP8))
    c16 = c8 = 0
    for (_, _, wname, v, _, gdt), c in zip(GROUPS, N_CH):
        we = np.concatenate(
            [np.asarray(inputs[wname], np.float32),
             np.ones((v, 1), np.float32)], axis=1)
        if c * P > v:
            we = np.concatenate(
                [we, np.zeros((c * P - v, FE), np.float32)], axis=0)
        chunked = we.reshape(c, P, FE).transpose(1, 0, 2)
        if gdt == _FP16:
            w16[:, c16:c16 + c, :] = chunked.astype(np.float16)
            c16 += c
        else:
            w8[:, c8:c8 + c, :] = chunked.astype(mybir.dt.np(_FP8))
            c8 += c
    return {
        "w16": np.ascontiguousarray(w16.reshape(P, C_16 * FE)),
        "w8": np.ascontiguousarray(w8.reshape(P, C_8 * FE)),
    }


def _pack_a(inputs: dict) -> np.ndarray:
    """Host transpose (int32 preserved): per core, vocab chunks on partitions.

    Returns [N_CORES, 128, C_TOT, 2*P] int32 where [core, p, c0g+c, b] =
    A_g[core*256 + b, c*128 + p] (zero beyond each group's vocab)."""
    arr = np.zeros((N_CORES, P, C_TOT, 2 * P), np.int32)
    c0 = 0
    for (_, aname, _, v, _, _), c in zip(GROUPS, N_CH):
        a = np.asarray(inputs[aname], np.int32).reshape(N_CORES, 2 * P, v)
        fc = v // P
        if fc:
            arr[:, :, c0:c0 + fc, :] = (
                a[:, :, :fc * P].reshape(N_CORES, 2 * P, fc, P)
                .transpose(0, 3, 2, 1))
        if v % P:
            arr[:, :v % P, c0 + fc, :] = a[:, :, fc * P:].transpose(0, 2, 1)
        c0 += c
    return arr


def _build_in_maps(inputs: dict) -> list[dict[str, np.ndarray]]:
    w_packs = _pack_weights(inputs)
    a_all = _pack_a(inputs)
    in_maps = []
    for core in range(N_CORES):
        m = dict(w_packs)
        m["a_all"] = a_all[core].reshape(P, C_TOT * 2 * P)
        in_maps.append(m)
    return in_maps


def kernel(**inputs: np.ndarray) -> np.ndarray:
    import os

    nc = _get_nc()
    in_maps = _build_in_maps(inputs)

    trace = bool(int(os.environ.get("EMB_TRACE", "0")))
    res = run_bass_kernel_spmd(nc, in_maps, core_ids=list(range(N_CORES)),
                               trace=trace)
    if trace and res.exec_time_ns is not None:
        print(f"HW exec time: {res.exec_time_ns} ns")
        if res.instructions_and_trace is not None:
            print(f"trace: {res.instructions_and_trace[1]}")

    return np.concatenate([r["out"] for r in res.results], axis=0)
